# revision 12
# baseline (speedup 1.0000x reference)
"""Trainium2 Bass kernel for the top-k hinge loss (nn_Loss3).

Math (per row b of x [B, C]):
    shifted = x + 1 everywhere except at the label y[b] (stays x[b, y[b]])
    ret[b]  = sum(relu(top5(shifted) - s_y)),   s_y = x[b, y[b]]
    out     = mean(ret / k)

Device algorithm:
    v8 = top-8 values of row (DVE Max8, multiset, sorted desc)
    s_y gathered via indirect DMA
    c  = #{v_i > s_y}
    top-5 of non-label row = first 5 of (v8 with index-c element dropped when c<5)
    ret = sum over kept i of relu(v_i - (s_y - 1))

x streams HBM->SBUF through the SWDGE ring with an in-DMA cast to bf16
(default): HBM reads are unchanged (the binding constraint, ~350 GB/s/NC)
but SBUF writes and DVE reads halve. The count/drop trick then compares
against round-trip bf16(s_y) so the label copy inside v8 still matches
bitwise; hinge values keep exact s_y - 1. Measured rel_err ~3e-5.

Sharding: data-parallel over rows, 1024 rows per core on 8 cores; host
averages the 8192 per-row sums.
"""

import time

import numpy as np

import concourse.bass as bass
import concourse.mybir as mybir
from concourse import bacc, tile
from concourse.bass_utils import run_bass_kernel_spmd

B, C = 8192, 50257
K = 5
N_CORES = 8
ROWS = B // N_CORES          # 1024 rows per core
P = 128                      # SBUF partitions
RT = ROWS // P               # 8 row-tiles per core

F32 = mybir.dt.float32
U32 = mybir.dt.uint32
I32 = mybir.dt.int32
Alu = mybir.AluOpType


def _chunk_sizes(cols, max_chunk=16384):
    n = -(-cols // max_chunk)
    base = cols // n
    rem = cols - base * n
    return [base + (1 if i < rem else 0) for i in range(n)]


def build_nc(
    rows=ROWS,
    cols=C,
    big_bufs=None,
    repeats=1,
    alt_rings=False,
    dma_splits=4,
    ring_pattern=None,
    cast16=True,
):
    """dma_splits: number of DMA loads per 128-row tile (cols split evenly).
    Each load is further cut into <=16384-wide InstMax calls. ring_pattern:
    string over {'s','a','g'} cycled across DMA loads (sync HWDGE,
    scalar/ACT HWDGE, gpsimd SWDGE); default alternates the two HWDGE rings.
    cast16: DMA-cast x to bf16 on load (SWDGE-only -> forces ring 'g');
    halves SBUF write traffic and DVE read traffic, HBM reads unchanged.
    """
    rt = rows // P
    dma_chunks = _chunk_sizes(cols, max_chunk=-(-cols // dma_splits))
    # per-DMA-load list of InstMax sub-widths
    max_chunks = [_chunk_sizes(w) for w in dma_chunks]
    nchunks = sum(len(m) for m in max_chunks)
    # Default to alternating the two HWDGE rings (SP + ACT): while one
    # ring's DMA waits out its completion-receipt round trip, the other
    # streams — measured ~6 us/repeat faster than single-ring.
    if ring_pattern is None:
        ring_pattern = "sa"
    if cast16:
        ring_pattern = "g"
    if big_bufs is None:
        big_bufs = 6 if cast16 else 3
    BF16 = mybir.dt.bfloat16
    xdt = BF16 if cast16 else F32

    nc = bacc.Bacc(None, target_bir_lowering=False)
    x_in = nc.dram_tensor("x", [rows, cols], F32, kind="ExternalInput")
    yoff = nc.dram_tensor("yoff", [rt, P, 1], U32, kind="ExternalInput")
    # [P, rt]: ret for row t*128+p lives at [p, t]; host transposes.
    ret_out = nc.dram_tensor("ret", [P, rt], F32, kind="ExternalOutput")

    x_flat = x_in.rearrange("r c -> (r c)")[:, None]

    with tile.TileContext(nc) as tc:
        with (
            tc.tile_pool(name="const", bufs=1) as cpool,
            tc.tile_pool(name="big", bufs=big_bufs) as bpool,
            tc.tile_pool(name="small", bufs=2) as spool,
        ):
            # All label gathers up front, on the SWDGE (gpsimd) queue so their
            # tiny descriptors never interleave with the streaming chunk loads
            # on the HWDGE ring. [128,1]-offset gathers are HW-verified; the
            # batched [128,rt]-offset form miscomputes on HW (sim divergence).
            yo_all = cpool.tile([P, rt], U32)
            nc.gpsimd.dma_start(
                out=yo_all[:], in_=yoff.rearrange("t p o -> p (t o)")
            )
            sy_all = cpool.tile([P, rt], F32)
            for t in range(rt):
                nc.gpsimd.indirect_dma_start(
                    out=sy_all[:, t : t + 1],
                    out_offset=None,
                    in_=x_flat[:],
                    in_offset=bass.IndirectOffsetOnAxis(
                        ap=yo_all[:, t : t + 1], axis=0
                    ),
                )
            a_all = cpool.tile([P, rt], F32)
            nc.vector.tensor_scalar(
                out=a_all[:], in0=sy_all[:], scalar1=1.0, scalar2=None, op0=Alu.subtract
            )
            # cast16: the count/drop trick needs bitwise equality between the
            # top-8 copy of the label value and the compared s_y, so compare
            # against round-trip bf16(s_y) instead of exact s_y. (a_all keeps
            # exact s_y-1 for the hinge values.)
            if cast16:
                sy_bf_all = cpool.tile([P, rt], BF16)
                nc.scalar.copy(out=sy_bf_all[:], in_=sy_all[:])
                sy_rt_all = cpool.tile([P, rt], F32)
                nc.scalar.copy(out=sy_rt_all[:], in_=sy_bf_all[:])
            else:
                sy_rt_all = sy_all
            ret_all = cpool.tile([P, rt], F32)

            engines = {"s": nc.sync, "a": nc.scalar, "g": nc.gpsimd}
            for t in range(rt * repeats):
                t = t % rt
                rows_slice = slice(t * P, (t + 1) * P)
                cand = spool.tile([P, 8 * nchunks], xdt, tag="cand")
                c0 = 0
                mi = 0
                for ci, w in enumerate(dma_chunks):
                    xt = bpool.tile([P, max(dma_chunks)], xdt, tag="xt")
                    eng = engines[ring_pattern[ci % len(ring_pattern)]]
                    eng.dma_start(
                        out=xt[:, :w], in_=x_in[rows_slice, c0 : c0 + w]
                    )
                    m0 = 0
                    for mw in max_chunks[ci]:
                        nc.vector.max(
                            out=cand[:, mi * 8 : (mi + 1) * 8],
                            in_=xt[:, m0 : m0 + mw],
                        )
                        m0 += mw
                        mi += 1
                    c0 += w
                v8 = spool.tile([P, 8], xdt, tag="v8")
                nc.vector.max(out=v8[:], in_=cand[:])
                if cast16:
                    v8f = spool.tile([P, 8], F32, tag="v8f")
                    nc.scalar.copy(out=v8f[:], in_=v8[:])
                    v8 = v8f

                sy = sy_rt_all[:, t : t + 1]
                a_t = a_all[:, t : t + 1]
                # Label exclusion: the top-5 of the non-label row sums to
                #   sum_{i<5} h_i + (c<5) * (h_5 - h_c)
                # where h_i = relu(v_i - a), c = #{v_i > s_y}. When c<5 the
                # dropped element v_c equals s_y bitwise, so h_c =
                # relu(s_y - a) — computable without indexing.
                gt = spool.tile([P, 8], F32, tag="gt")
                nc.vector.tensor_scalar(
                    out=gt[:], in0=v8[:], scalar1=sy, scalar2=None, op0=Alu.is_gt
                )
                c_t = spool.tile([P, 1], F32, tag="c")
                nc.vector.tensor_reduce(
                    out=c_t[:], in_=gt[:], axis=mybir.AxisListType.X, op=Alu.add
                )
                # h_i = relu(v_i - a), i in 0..5 (6 values suffice)
                h = spool.tile([P, 6], F32, tag="h")
                nc.vector.tensor_scalar(
                    out=h[:],
                    in0=v8[:, 0:6],
                    scalar1=a_t,
                    scalar2=0.0,
                    op0=Alu.subtract,
                    op1=Alu.max,
                )
                # h_c = relu(s_y - a)
                hc = spool.tile([P, 1], F32, tag="hc")
                nc.vector.tensor_scalar(
                    out=hc[:],
                    in0=sy,
                    scalar1=a_t,
                    scalar2=0.0,
                    op0=Alu.subtract,
                    op1=Alu.max,
                )
                # g5 = (c < 5)
                g5 = spool.tile([P, 1], F32, tag="g5")
                nc.vector.tensor_scalar(
                    out=g5[:], in0=c_t[:], scalar1=5.0, scalar2=None, op0=Alu.is_lt
                )
                # S5 = sum_{i<5} h_i
                s5 = spool.tile([P, 1], F32, tag="s5")
                nc.vector.tensor_reduce(
                    out=s5[:], in_=h[:, 0:5], axis=mybir.AxisListType.X, op=Alu.add
                )
                # d = h_5 - h_c;  ret = S5 + g5 * d
                d_t = spool.tile([P, 1], F32, tag="d")
                nc.vector.tensor_tensor(
                    out=d_t[:], in0=h[:, 5:6], in1=hc[:], op=Alu.subtract
                )
                gd = spool.tile([P, 1], F32, tag="gd")
                nc.vector.tensor_tensor(out=gd[:], in0=d_t[:], in1=g5[:], op=Alu.mult)
                nc.vector.tensor_tensor(
                    out=ret_all[:, t : t + 1], in0=s5[:], in1=gd[:], op=Alu.add
                )
            nc.sync.dma_start(out=ret_out[:], in_=ret_all[:])

    nc.compile()
    return nc


_NC = None


def _get_nc():
    global _NC
    if _NC is None:
        _NC = build_nc()
    return _NC


def make_in_maps(x, y):
    x = np.ascontiguousarray(np.asarray(x, dtype=np.float32))
    y = np.asarray(y).astype(np.int64)
    assert x.shape == (B, C), x.shape
    assert y.shape == (B,), y.shape
    in_maps = []
    local_r = np.arange(ROWS, dtype=np.int64)
    for core in range(N_CORES):
        r0 = core * ROWS
        y_loc = y[r0 : r0 + ROWS]
        off = local_r * C + y_loc
        assert off.max() < 2**32
        in_maps.append(
            {
                "x": x[r0 : r0 + ROWS],
                "yoff": off.astype(np.uint32).reshape(RT, P, 1),
            }
        )
    return in_maps


def finish(results, k):
    # device output is [P, RT] with row t*128+p at [p, t] -> transpose
    rets = np.concatenate(
        [
            np.asarray(r["ret"], dtype=np.float32).T.reshape(ROWS)
            for r in results
        ]
    )
    return np.asarray(np.mean(rets.astype(np.float64)) / k, dtype=np.float32)


def kernel(x, y, k):
    k = int(k)
    assert k == K, k
    nc = _get_nc()
    in_maps = make_in_maps(x, y)
    last_err = None
    for attempt in range(3):
        try:
            res = run_bass_kernel_spmd(nc, in_maps, core_ids=list(range(N_CORES)))
            return finish(res.results, k)
        except Exception as e:  # transient device-unrecoverable states heal
            last_err = e
            time.sleep(15 * (attempt + 1))
    raise last_err



# revision 14
# speedup vs baseline: 1.2002x; 1.2002x over previous
"""Trainium2 Bass kernel for the top-k hinge loss (nn_Loss3).

Math (per row b of x [B, C]):
    shifted = x + 1 everywhere except at the label y[b] (stays x[b, y[b]])
    ret[b]  = sum(relu(top5(shifted) - s_y)),   s_y = x[b, y[b]]
    out     = mean(ret / k)

Device algorithm:
    v8 = top-8 values of row (DVE Max8, multiset, sorted desc)
    s_y gathered via indirect DMA
    c  = #{v_i > s_y}
    top-5 of non-label row = first 5 of (v8 with index-c element dropped when c<5)
    ret = sum over kept i of relu(v_i - (s_y - 1))

x streams HBM->SBUF as f32, alternating the two HWDGE rings (SP+ACT) so
one ring's completion-receipt round trip hides behind the other's stream.
DVE Max8 in f32 runs ~440 us/repeat, safely under the ~500 us HBM wall.
(A bf16 in-DMA-cast variant (cast16=True) halves SBUF/DVE bytes but bf16
Max8 is ~21% slower per element — 528 us/repeat — and becomes the
bottleneck, so f32 is the default.)

Sharding: data-parallel over rows, 1024 rows per core on 8 cores; host
averages the 8192 per-row sums.
"""

import time

import numpy as np

import concourse.bass as bass
import concourse.mybir as mybir
from concourse import bacc, tile
from concourse.bass_utils import run_bass_kernel_spmd

B, C = 8192, 50257
K = 5
N_CORES = 8
ROWS = B // N_CORES          # 1024 rows per core
P = 128                      # SBUF partitions
RT = ROWS // P               # 8 row-tiles per core

F32 = mybir.dt.float32
U32 = mybir.dt.uint32
I32 = mybir.dt.int32
Alu = mybir.AluOpType


def _chunk_sizes(cols, max_chunk=16384):
    n = -(-cols // max_chunk)
    base = cols // n
    rem = cols - base * n
    return [base + (1 if i < rem else 0) for i in range(n)]


def build_nc(
    rows=ROWS,
    cols=C,
    big_bufs=None,
    repeats=1,
    alt_rings=False,
    dma_splits=4,
    ring_pattern=None,
    cast16=False,
):
    """dma_splits: number of DMA loads per 128-row tile (cols split evenly).
    Each load is further cut into <=16384-wide InstMax calls. ring_pattern:
    string over {'s','a','g'} cycled across DMA loads (sync HWDGE,
    scalar/ACT HWDGE, gpsimd SWDGE); default alternates the two HWDGE rings.
    cast16: DMA-cast x to bf16 on load (SWDGE-only -> forces ring 'g');
    halves SBUF write traffic and DVE read traffic, HBM reads unchanged.
    """
    rt = rows // P
    dma_chunks = _chunk_sizes(cols, max_chunk=-(-cols // dma_splits))
    # per-DMA-load list of InstMax sub-widths
    max_chunks = [_chunk_sizes(w) for w in dma_chunks]
    nchunks = sum(len(m) for m in max_chunks)
    # Default to alternating the two HWDGE rings (SP + ACT): while one
    # ring's DMA waits out its completion-receipt round trip, the other
    # streams — measured ~6 us/repeat faster than single-ring.
    if ring_pattern is None:
        ring_pattern = "sa"
    if cast16:
        ring_pattern = "g"
    if big_bufs is None:
        big_bufs = 6 if cast16 else 3
    BF16 = mybir.dt.bfloat16
    xdt = BF16 if cast16 else F32

    nc = bacc.Bacc(None, target_bir_lowering=False)
    x_in = nc.dram_tensor("x", [rows, cols], F32, kind="ExternalInput")
    yoff = nc.dram_tensor("yoff", [rt, P, 1], U32, kind="ExternalInput")
    # [P, rt]: ret for row t*128+p lives at [p, t]; host transposes.
    ret_out = nc.dram_tensor("ret", [P, rt], F32, kind="ExternalOutput")

    x_flat = x_in.rearrange("r c -> (r c)")[:, None]

    with tile.TileContext(nc) as tc:
        with (
            tc.tile_pool(name="const", bufs=1) as cpool,
            tc.tile_pool(name="big", bufs=big_bufs) as bpool,
            tc.tile_pool(name="small", bufs=2) as spool,
        ):
            # All label gathers up front, on the SWDGE (gpsimd) queue so their
            # tiny descriptors never interleave with the streaming chunk loads
            # on the HWDGE ring. [128,1]-offset gathers are HW-verified; the
            # batched [128,rt]-offset form miscomputes on HW (sim divergence).
            yo_all = cpool.tile([P, rt], U32)
            nc.gpsimd.dma_start(
                out=yo_all[:], in_=yoff.rearrange("t p o -> p (t o)")
            )
            sy_all = cpool.tile([P, rt], F32)
            for t in range(rt):
                nc.gpsimd.indirect_dma_start(
                    out=sy_all[:, t : t + 1],
                    out_offset=None,
                    in_=x_flat[:],
                    in_offset=bass.IndirectOffsetOnAxis(
                        ap=yo_all[:, t : t + 1], axis=0
                    ),
                )
            a_all = cpool.tile([P, rt], F32)
            nc.vector.tensor_scalar(
                out=a_all[:], in0=sy_all[:], scalar1=1.0, scalar2=None, op0=Alu.subtract
            )
            # cast16: the count/drop trick needs bitwise equality between the
            # top-8 copy of the label value and the compared s_y, so compare
            # against round-trip bf16(s_y) instead of exact s_y. (a_all keeps
            # exact s_y-1 for the hinge values.)
            if cast16:
                sy_bf_all = cpool.tile([P, rt], BF16)
                nc.scalar.copy(out=sy_bf_all[:], in_=sy_all[:])
                sy_rt_all = cpool.tile([P, rt], F32)
                nc.scalar.copy(out=sy_rt_all[:], in_=sy_bf_all[:])
            else:
                sy_rt_all = sy_all
            ret_all = cpool.tile([P, rt], F32)

            engines = {"s": nc.sync, "a": nc.scalar, "g": nc.gpsimd}
            for t in range(rt * repeats):
                t = t % rt
                rows_slice = slice(t * P, (t + 1) * P)
                cand = spool.tile([P, 8 * nchunks], xdt, tag="cand")
                c0 = 0
                mi = 0
                for ci, w in enumerate(dma_chunks):
                    xt = bpool.tile([P, max(dma_chunks)], xdt, tag="xt")
                    eng = engines[ring_pattern[ci % len(ring_pattern)]]
                    eng.dma_start(
                        out=xt[:, :w], in_=x_in[rows_slice, c0 : c0 + w]
                    )
                    m0 = 0
                    for mw in max_chunks[ci]:
                        nc.vector.max(
                            out=cand[:, mi * 8 : (mi + 1) * 8],
                            in_=xt[:, m0 : m0 + mw],
                        )
                        m0 += mw
                        mi += 1
                    c0 += w
                v8 = spool.tile([P, 8], xdt, tag="v8")
                nc.vector.max(out=v8[:], in_=cand[:])
                if cast16:
                    v8f = spool.tile([P, 8], F32, tag="v8f")
                    nc.scalar.copy(out=v8f[:], in_=v8[:])
                    v8 = v8f

                sy = sy_rt_all[:, t : t + 1]
                a_t = a_all[:, t : t + 1]
                # Label exclusion: the top-5 of the non-label row sums to
                #   sum_{i<5} h_i + (c<5) * (h_5 - h_c)
                # where h_i = relu(v_i - a), c = #{v_i > s_y}. When c<5 the
                # dropped element v_c equals s_y bitwise, so h_c =
                # relu(s_y - a) — computable without indexing.
                gt = spool.tile([P, 8], F32, tag="gt")
                nc.vector.tensor_scalar(
                    out=gt[:], in0=v8[:], scalar1=sy, scalar2=None, op0=Alu.is_gt
                )
                c_t = spool.tile([P, 1], F32, tag="c")
                nc.vector.tensor_reduce(
                    out=c_t[:], in_=gt[:], axis=mybir.AxisListType.X, op=Alu.add
                )
                # h_i = relu(v_i - a), i in 0..5 (6 values suffice)
                h = spool.tile([P, 6], F32, tag="h")
                nc.vector.tensor_scalar(
                    out=h[:],
                    in0=v8[:, 0:6],
                    scalar1=a_t,
                    scalar2=0.0,
                    op0=Alu.subtract,
                    op1=Alu.max,
                )
                # h_c = relu(s_y - a)
                hc = spool.tile([P, 1], F32, tag="hc")
                nc.vector.tensor_scalar(
                    out=hc[:],
                    in0=sy,
                    scalar1=a_t,
                    scalar2=0.0,
                    op0=Alu.subtract,
                    op1=Alu.max,
                )
                # g5 = (c < 5)
                g5 = spool.tile([P, 1], F32, tag="g5")
                nc.vector.tensor_scalar(
                    out=g5[:], in0=c_t[:], scalar1=5.0, scalar2=None, op0=Alu.is_lt
                )
                # S5 = sum_{i<5} h_i
                s5 = spool.tile([P, 1], F32, tag="s5")
                nc.vector.tensor_reduce(
                    out=s5[:], in_=h[:, 0:5], axis=mybir.AxisListType.X, op=Alu.add
                )
                # d = h_5 - h_c;  ret = S5 + g5 * d
                d_t = spool.tile([P, 1], F32, tag="d")
                nc.vector.tensor_tensor(
                    out=d_t[:], in0=h[:, 5:6], in1=hc[:], op=Alu.subtract
                )
                gd = spool.tile([P, 1], F32, tag="gd")
                nc.vector.tensor_tensor(out=gd[:], in0=d_t[:], in1=g5[:], op=Alu.mult)
                nc.vector.tensor_tensor(
                    out=ret_all[:, t : t + 1], in0=s5[:], in1=gd[:], op=Alu.add
                )
            nc.sync.dma_start(out=ret_out[:], in_=ret_all[:])

    nc.compile()
    return nc


_NC = None


def _get_nc():
    global _NC
    if _NC is None:
        _NC = build_nc()
    return _NC


def make_in_maps(x, y):
    x = np.ascontiguousarray(np.asarray(x, dtype=np.float32))
    y = np.asarray(y).astype(np.int64)
    assert x.shape == (B, C), x.shape
    assert y.shape == (B,), y.shape
    in_maps = []
    local_r = np.arange(ROWS, dtype=np.int64)
    for core in range(N_CORES):
        r0 = core * ROWS
        y_loc = y[r0 : r0 + ROWS]
        off = local_r * C + y_loc
        assert off.max() < 2**32
        in_maps.append(
            {
                "x": x[r0 : r0 + ROWS],
                "yoff": off.astype(np.uint32).reshape(RT, P, 1),
            }
        )
    return in_maps


def finish(results, k):
    # device output is [P, RT] with row t*128+p at [p, t] -> transpose
    rets = np.concatenate(
        [
            np.asarray(r["ret"], dtype=np.float32).T.reshape(ROWS)
            for r in results
        ]
    )
    return np.asarray(np.mean(rets.astype(np.float64)) / k, dtype=np.float32)


def kernel(x, y, k):
    k = int(k)
    assert k == K, k
    nc = _get_nc()
    in_maps = make_in_maps(x, y)
    last_err = None
    for attempt in range(3):
        try:
            res = run_bass_kernel_spmd(nc, in_maps, core_ids=list(range(N_CORES)))
            return finish(res.results, k)
        except Exception as e:  # transient device-unrecoverable states heal
            last_err = e
            time.sleep(15 * (attempt + 1))
    raise last_err

